# revision 13
# baseline (speedup 1.0000x reference)
"""Trainium2 Bass kernel for nn_EwaldBlock (gnn_message_passing).

Strategy: shard by GRAPH (B=32 graphs -> 4 per core, batch-contiguous), so the
per-graph structure factors sf_real/sf_imag are computed entirely on one core
and no collective is needed.  Each graph is padded to a whole number of
128-node tiles; slot sizes are shared across cores (SPMD: one program, per-core
input shards).  Inside a core everything is expressed as matmuls on the PE plus
elementwise work spread across ACT/DVE/GPSIMD:

  x (feature-major, bf16)  --W_pre1/W_pre2 matmuls + Silu-->  h
  xres = x + h  --PE transpose-->  node-major  --bn_stats LN-->  xln (bf16)
  trig = [cos,sin](k_dot_r)*sinc  (ACT Sin with exact range reduction)
  sfT[d,2K]  = xln^T @ trig            (one matmul chain per graph, fp32 PSUM)
  srsi[2K,d] = transpose(sfT) * (kfilter*gamma) (kfilter = dp @ W_up^T on PE)
  msgT[d,n]  = srsi^T-matmul trigT     (trigT via PE transposes)
  x2 = x(fp32) + msg ; out = x2 + MLP2(x2)   (residuals in fp32)

Host side: shard/pad/transpose inputs per core, run via run_bass_kernel_spmd
on 8 cores, gather + unpad the full [16384,128] fp32 output.
"""

from contextlib import ExitStack

import numpy as np
import ml_dtypes

import concourse.bass as bass
import concourse.tile as tile
from concourse import mybir
from concourse.bass_utils import run_bass_kernel_spmd
from concourse.masks import make_identity

BF16 = mybir.dt.bfloat16
F32 = mybir.dt.float32
F32R = mybir.dt.float32r
AF = mybir.ActivationFunctionType
ALU = mybir.AluOpType

N_CORES = 8
D = 128
K = 64
TWO_K = 2 * K
LN_EPS = 1e-5
PI = float(np.pi)
RN_C = 12582912.0          # 1.5*2^23: (x + C) - C == round-to-nearest(x), fp32
INV_2PI = float(1.0 / (2.0 * np.pi))

CONFIG = {
    "act_mode": "silu",    # "silu" (HW) | "sigmoid_mul" (CoreSim-compatible)
    "split_waits": True,   # walrus needs <=1 wait/inst; CoreSim can't run nops
}

TRACE = False            # set by test harness for profiling
LAST_EXEC_NS = None
LAST_RESULTS = None

_PROGRAM_CACHE = {}


# --------------------------------------------------------------------------
# device program
# --------------------------------------------------------------------------

def _pieces(w, maxw=512):
    p = 0
    while p < w:
        pw = min(maxw, w - p)
        yield p, pw
        p += pw


_SPLIT_TYPES = (
    "InstTensorTensor", "InstTensorScalarPtr", "InstTensorCopy",
    "InstReciprocal", "InstBNStats", "InstBNStatsAggregate",
    "InstActivation", "InstMemset", "InstIota", "InstTensorReduce",
    "InstMatmult", "InstLdweights", "InstTensorScalarAffineSelect",
    "InstCopyPredicated", "InstDMACopy", "InstDrain",
)


def _split_excess_waits(nc, limit=1):
    """walrus's per-instruction ISA structs hold few sync waits (the DVE
    TensorTensor struct rejects >1).  Move excess waits onto same-engine
    NoOps inserted immediately before the instruction."""
    n_id = 0
    for f in nc.m.functions:
        for bb in f.blocks:
            insts = bb.instructions
            out = []
            for inst in insts:
                si = inst.sync_info
                if (si is not None and si.on_wait
                        and len(si.on_wait) > limit
                        and type(inst).__name__ in _SPLIT_TYPES):
                    waits = list(si.on_wait)
                    extra, keep = waits[:-limit], waits[-limit:]
                    for wchunk in [extra[i:i + limit]
                                   for i in range(0, len(extra), limit)]:
                        nop = mybir.InstNoOp(name=f"I-waitnop-{n_id}")
                        n_id += 1
                        nop.engine = inst.engine
                        nop.sync_info = mybir.SyncInfo(
                            on_wait=list(wchunk), on_update=[])
                        out.append(nop)
                    inst.sync_info = mybir.SyncInfo(
                        on_wait=keep, on_update=list(si.on_update))
                out.append(inst)
            insts[:] = out
    return nc


def build_program(slot_T):
    """Build the SPMD Bass program for per-core slot tile counts slot_T."""
    slot_T = tuple(int(t) for t in slot_T)
    G = len(slot_T)
    n_pad = 128 * sum(slot_T)
    k_cols = 64 * sum(slot_T)

    nc = bass.Bass()

    # ---- DRAM parameters (per-core shards via in_maps) -------------------
    xt32_d = nc.declare_dram_parameter("xt32", [D, n_pad], F32, isOutput=False)
    xtbf_d = nc.declare_dram_parameter("xtbf", [D, n_pad], BF16, isOutput=False)
    kdr_d = nc.declare_dram_parameter("kdr", [128, k_cols], F32, isOutput=False)
    sinc_d = nc.declare_dram_parameter("sinc", [128, k_cols], BF16, isOutput=False)
    w1t_d = nc.declare_dram_parameter("w1t", [D, D], BF16, isOutput=False)
    w2t_d = nc.declare_dram_parameter("w2t", [D, D], BF16, isOutput=False)
    wu1t_d = nc.declare_dram_parameter("wu1t", [D, D], BF16, isOutput=False)
    wu2t_d = nc.declare_dram_parameter("wu2t", [D, D], BF16, isOutput=False)
    dpt_d = nc.declare_dram_parameter("dpt", [8, K], BF16, isOutput=False)
    wupt_d = nc.declare_dram_parameter("wupt", [8, D], BF16, isOutput=False)
    gam_d = nc.declare_dram_parameter("gam", [D, D], F32, isOutput=False)
    out_d = nc.declare_dram_parameter("outt", [D, n_pad], F32, isOutput=True)

    act_silu = CONFIG["act_mode"] == "silu"

    with tile.TileContext(nc) as tc, ExitStack() as ctx:
        consts = ctx.enter_context(tc.tile_pool(name="consts", bufs=1))
        pers = ctx.enter_context(tc.tile_pool(name="pers", bufs=1))
        work = ctx.enter_context(tc.tile_pool(name="work", bufs=3))
        ps = ctx.enter_context(tc.tile_pool(name="ps", bufs=4, space="PSUM"))
        trps = ctx.enter_context(tc.tile_pool(name="trps", bufs=2, space="PSUM"))
        sfps = ctx.enter_context(tc.tile_pool(name="sfps", bufs=2, space="PSUM"))

        # ---- constants / weights ----------------------------------------
        w1t = consts.tile([D, D], BF16)
        nc.sync.dma_start(out=w1t, in_=w1t_d[:, :])
        w2t = consts.tile([D, D], BF16)
        nc.sync.dma_start(out=w2t, in_=w2t_d[:, :])
        wu1t = consts.tile([D, D], BF16)
        nc.sync.dma_start(out=wu1t, in_=wu1t_d[:, :])
        wu2t = consts.tile([D, D], BF16)
        nc.sync.dma_start(out=wu2t, in_=wu2t_d[:, :])
        dpt = consts.tile([8, K], BF16)
        nc.sync.dma_start(out=dpt, in_=dpt_d[:, :])
        wupt = consts.tile([8, D], BF16)
        nc.sync.dma_start(out=wupt, in_=wupt_d[:, :])
        gam = consts.tile([D, D], F32)
        nc.sync.dma_start(out=gam, in_=gam_d[:, :])

        ident = consts.tile([D, D], BF16)
        make_identity(nc, ident)

        for i, cv in enumerate([0.0, LN_EPS, PI / 2.0]):
            cvt = consts.tile([128, 1], F32, name=f"constap{i}")
            nc.vector.memset(cvt, cv)
            nc.const_aps.aps[(F32, float(cv))] = cvt

        # kfilter*gamma, replicated [2K, D]
        kf_p = sfps.tile([K, D], F32, tag="sf")
        nc.tensor.matmul(kf_p, dpt, wupt, start=True, stop=True)
        kf_sb = consts.tile([K, D], F32)
        nc.scalar.activation(kf_sb, kf_p, AF.Copy)
        kfr = consts.tile([TWO_K, D], BF16)
        nc.vector.tensor_mul(kfr[0:K, :], kf_sb, gam[0:K, :])
        nc.sync.dma_start(out=kfr[K:TWO_K, :], in_=kfr[0:K, :])

        # ---- persistent per-core tensors --------------------------------
        xt32 = pers.tile([D, n_pad], F32)
        nc.sync.dma_start(out=xt32, in_=xt32_d[:, :])
        xtbf = pers.tile([D, n_pad], BF16)
        nc.sync.dma_start(out=xtbf, in_=xtbf_d[:, :])
        kdr_all = pers.tile([128, k_cols], F32)
        nc.sync.dma_start(out=kdr_all, in_=kdr_d[:, :])
        sinc_all = pers.tile([128, k_cols], BF16)
        nc.sync.dma_start(out=sinc_all, in_=sinc_d[:, :])
        trigT_all = pers.tile([TWO_K, n_pad], BF16)

        def act(dst, src_psum):
            """dst_sbuf = silu(src_psum)"""
            if act_silu:
                nc.scalar.activation(dst, src_psum, AF.Silu)
            else:
                sg = work.tile(list(dst.shape), BF16, name="sgm", tag="sgm")
                nc.scalar.activation(sg, src_psum, AF.Sigmoid)
                nc.vector.tensor_mul(dst, src_psum, sg)

        def mlp_layer(dst, lhsT, rhs, w, col0):
            """dst_sbuf[:, 0:w] = silu(lhsT.T @ rhs[:, col0:col0+w]), piecewise."""
            for p, pw in _pieces(w):
                mp = ps.tile([D, 512], F32, name="mp", tag="ps")
                nc.tensor.matmul(mp[:, 0:pw], lhsT,
                                 rhs[:, col0 + p:col0 + p + pw],
                                 start=True, stop=True)
                act(dst[:, p:p + pw], mp[:, 0:pw])

        col = 0
        kcol = 0
        for j in range(G):
            Tj = slot_T[j]
            w = 128 * Tj
            kw = 64 * Tj

            # ---- MLP1: h = silu(silu(x@W1^T)@W2^T), feature-major -------
            h1 = work.tile([D, w], BF16, tag="h1")
            mlp_layer(h1, w1t, xtbf, w, col)
            h2 = work.tile([D, w], BF16, tag="h2")
            mlp_layer(h2, w2t, h1, w, 0)
            xres = work.tile([D, w], BF16, tag="xres")
            nc.gpsimd.tensor_add(xres, xtbf[:, col:col + w], h2)

            # ---- transpose to node-major, LayerNorm ---------------------
            xrnm_p = trps.tile([128, w], BF16, name="xrnm_p", tag="tr")
            for t in range(Tj):
                nc.tensor.transpose(xrnm_p[:, t * 128:(t + 1) * 128],
                                    xres[:, t * 128:(t + 1) * 128], ident)
            xrnm = work.tile([128, w], BF16, tag="xrnm")
            nc.scalar.activation(xrnm, xrnm_p, AF.Copy)

            st6 = work.tile([128, Tj, 6], F32, tag="st6")
            mv = work.tile([128, Tj, 2], F32, tag="mv")
            for t in range(Tj):
                nc.vector.bn_stats(st6[:, t, :], xrnm[:, t * 128:(t + 1) * 128])
                nc.vector.bn_aggr(mv[:, t, :], st6[:, t, :])
            sd = work.tile([128, Tj], F32, tag="sd")
            nc.scalar.activation(sd, mv[:, :, 1], AF.Sqrt, bias=LN_EPS)
            rstd = work.tile([128, Tj], F32, tag="rstd")
            nc.vector.reciprocal(rstd, sd)
            xln = work.tile([128, w], BF16, tag="xln")
            for t in range(Tj):
                nc.vector.tensor_scalar(
                    out=xln[:, t * 128:(t + 1) * 128],
                    in0=xrnm[:, t * 128:(t + 1) * 128],
                    scalar1=mv[:, t, 0:1], scalar2=rstd[:, t:t + 1],
                    op0=ALU.subtract, op1=ALU.mult)

            # ---- trig: cos/sin(k_dot_r)*sinc, node-major ----------------
            kdr = kdr_all[:, kcol:kcol + kw]
            k1 = work.tile([128, kw], F32, tag="k1")
            nc.scalar.mul(k1, kdr, INV_2PI)
            kr = work.tile([128, kw], F32, tag="kr")
            nc.vector.tensor_scalar(out=kr, in0=k1, scalar1=RN_C,
                                    scalar2=RN_C, op0=ALU.add, op1=ALU.subtract)
            rs = work.tile([128, kw], F32, tag="rs")
            nc.vector.scalar_tensor_tensor(out=rs, in0=kr, scalar=-2.0 * PI,
                                           in1=kdr, op0=ALU.mult, op1=ALU.add)
            rc = work.tile([128, kw], F32, tag="rc")   # |r| for cos path
            nc.vector.scalar_tensor_tensor(out=rc, in0=rs, scalar=-1.0,
                                           in1=rs, op0=ALU.mult, op1=ALU.max)
            rs2 = work.tile([128, kw], F32, tag="rs2")  # clamp r to [-pi,pi]
            nc.vector.tensor_scalar(out=rs2, in0=rs, scalar1=PI, scalar2=-PI,
                                    op0=ALU.min, op1=ALU.max)
            cs = work.tile([128, Tj, TWO_K], BF16, tag="cs")
            cs3 = cs
            rs3 = rs2.rearrange("p (t k) -> p t k", k=64)
            rc3 = rc.rearrange("p (t k) -> p t k", k=64)
            # cos(x) = sin(pi/2 - |r|);  sin(x) = sin(r)
            nc.scalar.activation(cs3[:, :, 0:K], rc3, AF.Sin,
                                 bias=PI / 2.0, scale=-1.0)
            nc.scalar.activation(cs3[:, :, K:TWO_K], rs3, AF.Sin)
            trig = work.tile([128, Tj, TWO_K], BF16, tag="trig")
            sinc3 = sinc_all[:, kcol:kcol + kw].rearrange("p (t k) -> p t k", k=64)
            nc.gpsimd.tensor_mul(trig[:, :, 0:K], cs3[:, :, 0:K], sinc3)
            nc.gpsimd.tensor_mul(trig[:, :, K:TWO_K], cs3[:, :, K:TWO_K], sinc3)

            # ---- structure factors sfT[d, 2K] (fp32 accumulate) ---------
            sf_p = sfps.tile([D, TWO_K], F32, name="sf_p", tag="sf")
            for t in range(Tj):
                nc.tensor.matmul(sf_p, xln[:, t * 128:(t + 1) * 128],
                                 trig[:, t, :], start=(t == 0),
                                 stop=(t == Tj - 1))
            sf_sb = work.tile([D, TWO_K], BF16, tag="sf_sb")
            nc.vector.tensor_copy(sf_sb, sf_p)
            srsi_p = trps.tile([TWO_K, D], BF16, name="srsi_p", tag="tr")
            nc.tensor.transpose(srsi_p[:, 0:D], sf_sb, ident)
            srsi = work.tile([TWO_K, D], BF16, tag="srsi")
            nc.vector.tensor_mul(srsi, srsi_p[:, 0:D], kfr)

            # ---- trigT (feature-major trig) -----------------------------
            trT_p = trps.tile([TWO_K, w], BF16, name="trT_p", tag="tr")
            for t in range(Tj):
                nc.tensor.transpose(trT_p[:, t * 128:(t + 1) * 128],
                                    trig[:, t, :], ident)
            nc.vector.tensor_copy(trigT_all[:, col:col + w], trT_p)

            # ---- message + residual -------------------------------------
            x2 = work.tile([D, w], F32, tag="x2")
            for p, pw in _pieces(w):
                mg = ps.tile([D, 512], F32, name="mg", tag="ps")
                nc.tensor.matmul(mg[:, 0:pw], srsi,
                                 trigT_all[:, col + p:col + p + pw],
                                 start=True, stop=True)
                nc.vector.tensor_add(x2[:, p:p + pw],
                                     xt32[:, col + p:col + p + pw],
                                     mg[:, 0:pw])

            # ---- MLP2 + final residual ----------------------------------
            x2bf = work.tile([D, w], BF16, tag="x2bf")
            nc.gpsimd.tensor_copy(x2bf, x2)
            u1 = work.tile([D, w], BF16, tag="u1")
            mlp_layer(u1, wu1t, x2bf, w, 0)
            u2 = work.tile([D, w], BF16, tag="u2")
            mlp_layer(u2, wu2t, u1, w, 0)
            outt = work.tile([D, w], F32, tag="outt")
            nc.vector.tensor_add(outt, x2, u2)
            nc.sync.dma_start(out=out_d[:, col:col + w], in_=outt)

            col += w
            kcol += kw

    if CONFIG["split_waits"]:
        _split_excess_waits(nc)
    return nc


# --------------------------------------------------------------------------
# host side
# --------------------------------------------------------------------------

def _shard(batch, n_graphs):
    """Graph segments + serpentine graph->core/slot assignment."""
    bounds = np.searchsorted(batch, np.arange(n_graphs + 1))
    sizes = np.diff(bounds)
    order = np.argsort(-sizes, kind="stable")
    g_per_core = n_graphs // N_CORES
    gid = np.empty((N_CORES, g_per_core), dtype=np.int64)
    for j in range(g_per_core):
        sl = order[j * N_CORES:(j + 1) * N_CORES]
        if j % 2 == 1:
            sl = sl[::-1]
        gid[:, j] = sl
    slot_T = tuple(
        max(1, int(np.ceil(max(sizes[gid[c][j]] for c in range(N_CORES)) / 128)))
        for j in range(g_per_core))
    return bounds, gid, slot_T


def kernel(x_scalar, k_dot_r, sinc_damping, batch, down_projection,
           W_pre1, W_pre2, ln_gamma, ln_beta, W_up, W_upd1, W_upd2):
    x_scalar = np.asarray(x_scalar, dtype=np.float32)
    k_dot_r = np.asarray(k_dot_r, dtype=np.float32)
    sinc_damping = np.asarray(sinc_damping, dtype=np.float32)
    batch = np.asarray(batch).astype(np.int64)
    down_projection = np.asarray(down_projection, dtype=np.float32)
    W_pre1 = np.asarray(W_pre1, dtype=np.float32)
    W_pre2 = np.asarray(W_pre2, dtype=np.float32)
    ln_gamma = np.asarray(ln_gamma, dtype=np.float32)
    ln_beta = np.asarray(ln_beta, dtype=np.float32)
    W_up = np.asarray(W_up, dtype=np.float32)
    W_upd1 = np.asarray(W_upd1, dtype=np.float32)
    W_upd2 = np.asarray(W_upd2, dtype=np.float32)

    assert np.allclose(ln_beta, 0.0), "nonzero ln_beta not supported"

    n, d = x_scalar.shape
    n_graphs = int(batch.max()) + 1 if batch.size else 1
    n_graphs = max(n_graphs, N_CORES)
    # round up so every core gets the same number of graph slots
    while n_graphs % N_CORES:
        n_graphs += 1

    bounds, gid, slot_T = _shard(batch, n_graphs)
    g_per_core = n_graphs // N_CORES
    n_pad = 128 * sum(slot_T)
    k_cols = 64 * sum(slot_T)
    offs = np.cumsum([0] + [128 * t for t in slot_T])

    key = (slot_T, CONFIG["act_mode"], CONFIG["split_waits"])
    if key not in _PROGRAM_CACHE:
        _PROGRAM_CACHE[key] = build_program(slot_T)
    nc = _PROGRAM_CACHE[key]

    bf = ml_dtypes.bfloat16
    shared = {
        "w1t": np.ascontiguousarray(W_pre1.T).astype(bf),
        "w2t": np.ascontiguousarray(W_pre2.T).astype(bf),
        "wu1t": np.ascontiguousarray(W_upd1.T).astype(bf),
        "wu2t": np.ascontiguousarray(W_upd2.T).astype(bf),
        "dpt": np.ascontiguousarray(down_projection.T).astype(bf),
        "wupt": np.ascontiguousarray(W_up.T).astype(bf),
        "gam": np.ascontiguousarray(np.tile(ln_gamma[None, :], (D, 1))),
    }

    in_maps = []
    for c in range(N_CORES):
        xp = np.zeros((n_pad, D), np.float32)
        kdrp = np.zeros((n_pad, K), np.float32)
        sincp = np.zeros((n_pad, K), np.float32)
        for j in range(g_per_core):
            g = gid[c][j]
            s, e = bounds[g], bounds[g + 1]
            xp[offs[j]:offs[j] + e - s] = x_scalar[s:e]
            kdrp[offs[j]:offs[j] + e - s] = k_dot_r[s:e]
            sincp[offs[j]:offs[j] + e - s] = sinc_damping[s:e]

        # node-major [n_pad, K] -> per-slot [128, T*64] shuffled layout
        def shuf(a):
            blocks = []
            for j in range(g_per_core):
                t = slot_T[j]
                blk = a[offs[j]:offs[j + 1]].reshape(t, 128, K)
                blocks.append(np.transpose(blk, (1, 0, 2)).reshape(128, t * K))
            return np.ascontiguousarray(np.concatenate(blocks, axis=1))

        xt = np.ascontiguousarray(xp.T)
        in_maps.append(dict(shared,
                            xt32=xt,
                            xtbf=xt.astype(bf),
                            kdr=shuf(kdrp),
                            sinc=shuf(sincp).astype(bf)))

    global LAST_EXEC_NS, LAST_RESULTS
    res = run_bass_kernel_spmd(nc, in_maps, list(range(N_CORES)), trace=TRACE)
    LAST_RESULTS = res
    LAST_EXEC_NS = getattr(res, "exec_time_ns", None)
    out = np.zeros((n, d), np.float32)
    for c in range(N_CORES):
        outT = np.asarray(res.results[c]["outt"], dtype=np.float32)
        for j in range(g_per_core):
            g = gid[c][j]
            s, e = bounds[g], bounds[g + 1]
            out[s:e] = outT[:, offs[j]:offs[j] + e - s].T
    return out
